# revision 13
# baseline (speedup 1.0000x reference)
"""BoT multi-head attention block (QKV proj + content/position attention +
out-proj + residual + LayerNorm) on 8 Trainium2 NeuronCores.

Sharding: tensor-parallel over heads (4 heads/core) x batch (2 batches, 4
cores each).  Each core computes q/k/v projections for its 256 feature
columns, full attention for its 4 heads, and a partial out-projection;
partials are summed with per-slice ReduceScatters over each 4-core batch
group (overlapped with attention of later slices), after which each core
applies residual + LayerNorm to its 4x128 rows.

Layout trick: attention logits are computed TRANSPOSED (j on partitions, i
free) so the softmax numerator matmul (P^T moving, V stationary) needs no
transpose of the probability matrix; an extra all-ones column in the
stationary V supplies the softmax denominator for free.  Host passes x and
pos pre-transposed.

Pipeline: residual+LayerNorm runs in a tail phase (never blocking the
in-order Vector queue mid-attention); out-proj for slice s is issued
between the two head-pair attention passes of slice s+1 so the PE queue
always has independent work while ReduceScatter s is in flight; a tiny
warm-up collective at t=0 absorbs the CC barrier + link warm-up.
"""

import contextlib
import os
import sys

os.environ.setdefault("MYCRO_LOCAL_CACHE", "1")
for _p in ("/opt/trn_rl_repo",):
    if os.path.isdir(_p) and _p not in sys.path:
        sys.path.append(_p)

import ml_dtypes
import numpy as np

import concourse.bass as bass
from concourse import bacc
import concourse.mybir as mybir
import concourse.tile as tile
from concourse.bass_utils import run_bass_kernel_spmd

FP = mybir.dt.float32
BF = mybir.dt.bfloat16
AF = mybir.ActivationFunctionType
ALU = mybir.AluOpType

B, N, D, H = 2, 2048, 1024, 16
NCORES = 8
GRP = 4                # cores per batch group
HPC = H // GRP         # heads per core = 4
C = D // GRP           # feature cols per core = 256
R = N // GRP           # output rows per core = 512
DH = D // H            # head dim = 64
SCALE = DH ** -0.5
LN_EPS = 1e-5

NT = N // 128          # 16 row tiles
KD = D // 128          # 8 contraction tiles over D
NS = N // 512          # 4 i-slices

ATT_DT = BF            # dtype of attention matmul operands
PROJ_DT = BF           # dtype of projection inputs (xT, wq/wk/wv)


def build():
    nc = bacc.Bacc("TRN2", target_bir_lowering=False, num_devices=NCORES)

    xT_t = nc.dram_tensor("xT", [D, N], PROJ_DT, kind="ExternalInput")
    posT_t = nc.dram_tensor("posT", [C, N], FP, kind="ExternalInput")
    wq_t = nc.dram_tensor("wq", [D, C], PROJ_DT, kind="ExternalInput")
    wk_t = nc.dram_tensor("wk", [D, C], PROJ_DT, kind="ExternalInput")
    wv_t = nc.dram_tensor("wv", [D, C], PROJ_DT, kind="ExternalInput")
    wo_t = nc.dram_tensor("wo", [C, D], BF, kind="ExternalInput")
    res_t = nc.dram_tensor("resid", [R, D], FP, kind="ExternalInput")
    g_t = nc.dram_tensor("ln_g", [D], FP, kind="ExternalInput")
    bt_t = nc.dram_tensor("ln_b", [D], FP, kind="ExternalInput")
    out_t = nc.dram_tensor("out", [R, D], FP, kind="ExternalOutput")

    res_tiles = res_t.ap().rearrange("(t p) d -> t p d", p=128)
    out_tiles = out_t.ap().rearrange("(t p) d -> t p d", p=128)

    def bcast_ap(ap, parts):
        return bass.AP(tensor=ap.tensor, offset=ap.offset,
                       ap=[[0, parts]] + list(ap.ap))

    with tile.TileContext(nc) as tc, contextlib.ExitStack() as ctx:
        persist = ctx.enter_context(tc.tile_pool(name="persist", bufs=1))
        attnp = ctx.enter_context(tc.tile_pool(name="attnp", bufs=1))
        psP = ctx.enter_context(tc.tile_pool(name="psP", bufs=1, space="PSUM"))
        psO = ctx.enter_context(tc.tile_pool(name="psO", bufs=3, space="PSUM"))
        psC = ctx.enter_context(tc.tile_pool(name="psC", bufs=2, space="PSUM"))
        dram = ctx.enter_context(tc.tile_pool(name="dram", bufs=1, space="DRAM"))

        ones64 = persist.tile([1, DH], FP, tag="ones64")
        nc.vector.memset(ones64, 1.0)
        onescol = persist.tile([128, 1], FP, tag="onescol")
        nc.vector.memset(onescol, 1.0)

        sbA = ctx.enter_context(tc.tile_pool(name="sbA", bufs=3))

        # ---------------- phase 1-2: load (pre-transposed on host), project
        ph12_ctx = contextlib.ExitStack()
        p12 = ph12_ctx.enter_context(tc.tile_pool(name="ph12", bufs=1))

        wq_sb = p12.tile([128, KD, C], PROJ_DT, tag="wq")
        wk_sb = p12.tile([128, KD, C], PROJ_DT, tag="wk")
        wv_sb = p12.tile([128, KD, C], PROJ_DT, tag="wv")
        xT_sb = p12.tile([128, KD, N], PROJ_DT, tag="xT")
        xT_src = xT_t.ap().rearrange("(k p) n -> p k n", p=128)
        wq_src = wq_t.ap().rearrange("(k p) c -> p k c", p=128)
        # interleave per-k chunks so the first projection matmul can start
        # as soon as wq[0]/xT[0] land instead of after the full staging DMA
        for k in range(KD):
            nc.sync.dma_start(out=wq_sb[:, k, :], in_=wq_src[:, k, :])
            nc.sync.dma_start(out=xT_sb[:, k, :], in_=xT_src[:, k, :])
        nc.sync.dma_start(out=wk_sb, in_=wk_t.ap().rearrange("(k p) c -> p k c", p=128))
        posT_sb = p12.tile([128, 2, N], FP, tag="posT")
        nc.sync.dma_start(out=posT_sb,
                          in_=posT_t.ap().rearrange("(m p) n -> p m n", p=128))
        nc.sync.dma_start(out=wv_sb, in_=wv_t.ap().rearrange("(k p) c -> p k c", p=128))
        xT = [xT_sb[:, k, :] for k in range(KD)]
        posT = [posT_sb[:, m, :] for m in range(2)]

        wo_sb = persist.tile([128, 2, D], BF, tag="wo")
        nc.sync.dma_start(out=wo_sb, in_=wo_t.ap().rearrange("(k p) d -> p k d", p=128))
        g_sb = persist.tile([128, D], FP, tag="g")
        b_sb = persist.tile([128, D], FP, tag="b")
        nc.gpsimd.dma_start(out=g_sb, in_=bcast_ap(g_t.ap(), 128))
        nc.gpsimd.dma_start(out=b_sb, in_=bcast_ap(bt_t.ap(), 128))

        # projections: qT/kpT [128 c, N] (head pair hp at rows 64*(h%2))
        qT = [attnp.tile([128, N], ATT_DT, name=f"qT{m}", tag=f"qT{m}") for m in range(2)]
        kpT = [attnp.tile([128, N], ATT_DT, name=f"kpT{m}", tag=f"kpT{m}") for m in range(2)]
        V = [attnp.tile([128, HPC, DH + 1], ATT_DT, name=f"V{t}", tag=f"V{t}")
             for t in range(NT)]

        def proj_qkp(m):
            for s in range(NS):
                q_ps = psP.tile([128, 512], FP, tag="ps", name="q_ps")
                for k in range(KD):
                    nc.tensor.matmul(q_ps, wq_sb[:, k, m * 128:(m + 1) * 128],
                                     xT[k][:, s * 512:(s + 1) * 512],
                                     start=(k == 0), stop=(k == KD - 1))
                nc.vector.tensor_copy(out=qT[m][:, s * 512:(s + 1) * 512], in_=q_ps)
            for s in range(NS):
                kp_ps = psP.tile([128, 512], FP, tag="ps", name="kp_ps")
                for k in range(KD):
                    nc.tensor.matmul(kp_ps, wk_sb[:, k, m * 128:(m + 1) * 128],
                                     xT[k][:, s * 512:(s + 1) * 512],
                                     start=(k == 0), stop=(k == KD - 1))
                nc.vector.tensor_add(out=kpT[m][:, s * 512:(s + 1) * 512],
                                     in0=kp_ps, in1=posT[m][:, s * 512:(s + 1) * 512])

        proj_qkp(0)
        for t in range(NT):
            v_ps = psP.tile([128, C], FP, tag="ps", name="v_ps")
            for k in range(KD):
                nc.tensor.matmul(v_ps, xT[k][:, t * 128:(t + 1) * 128], wv_sb[:, k, :],
                                 start=(k == 0), stop=(k == KD - 1))
            nc.vector.tensor_copy(out=V[t][:, :, 0:DH],
                                  in_=v_ps.rearrange("p (h d) -> p h d", h=HPC))
            nc.vector.tensor_copy(out=V[t][:, :, DH:DH + 1],
                                  in_=onescol.broadcast_to([128, HPC, 1]))

        # ---------------- phases 3-5: attention / out-proj+RS / tail LN ----
        pools = {}

        # unnormalized attention output, bf16, normalized in place on gpsimd
        OT = [attnp.tile([128, N], BF, name=f"OT{m}", tag=f"OT{m}") for m in range(2)]
        oph = [dram.tile([R, D], FP, name=f"oph{s}", tag=f"oph{s}") for s in range(NS)]
        rsh = [dram.tile([128, D], FP, name=f"rsh{s}", tag=f"rsh{s}") for s in range(NS)]

        def attention(s, hp):
            ot_e = psO.tile([128, 512], FP, tag="ot", name="ot_e")
            ot_o = psO.tile([128, 512], FP, tag="ot", name="ot_o")
            for jt in range(NT):
                st = psC.tile([128, 1024], FP, tag="st", name="st")
                nc.tensor.matmul(st[:, 0:512],
                                 kpT[hp][0:64, jt * 128:(jt + 1) * 128],
                                 qT[hp][0:64, s * 512:(s + 1) * 512],
                                 start=True, stop=True)
                nc.tensor.matmul(st[:, 512:1024],
                                 kpT[hp][64:128, jt * 128:(jt + 1) * 128],
                                 qT[hp][64:128, s * 512:(s + 1) * 512],
                                 start=True, stop=True)
                ste = sbA.tile([128, 1024], ATT_DT, tag="ste", name="ste")
                nc.scalar.activation(out=ste, in_=st, func=AF.Exp, scale=SCALE)
                nc.tensor.matmul(ot_e[0:DH + 1, :], V[jt][:, 2 * hp, :],
                                 ste[:, 0:512],
                                 start=(jt == 0), stop=(jt == NT - 1))
                nc.tensor.matmul(ot_o[0:DH + 1, :], V[jt][:, 2 * hp + 1, :],
                                 ste[:, 512:1024],
                                 start=(jt == 0), stop=(jt == NT - 1))
            # evacuate PSUM: reciprocal of the colsum row straight from PSUM,
            # unnormalized rows to fp32 staging; softmax division writes the
            # bf16 OT used as the out-proj stationary
            for par, ot in ((0, ot_e), (1, ot_o)):
                csrow = sbA.tile([1, 512], FP, tag="csrow", name="csrow", bufs=8)
                nc.vector.tensor_copy(out=csrow, in_=ot[DH:DH + 1, :])
                csr = sbA.tile([1, 512], FP, tag="csr", name="csr", bufs=4)
                nc.vector.reciprocal_approx_fast(out=csr, in_=csrow)
                otu = sbA.tile([128, 512], FP, tag="otu", name="otu", bufs=4)
                otus = otu[par * 64:par * 64 + DH, :]
                nc.vector.tensor_copy(out=otus, in_=ot[0:DH, :])
                dst = OT[hp][par * 64:par * 64 + DH, s * 512:(s + 1) * 512]
                cs_d = dram.tile([1, 512], FP, tag="cs_d", name="cs_d", bufs=4)
                nc.sync.dma_start(out=cs_d[:], in_=csr)
                # rec must share its base partition with dst (DVE 2-SBUF rule)
                rec = sbA.tile([128, 512], FP, tag="rec", name="rec", bufs=4)
                recs = rec[par * 64:par * 64 + DH, :]
                cs_d_ap = cs_d.opt()
                nc.gpsimd.dma_start(out=recs, in_=bass.AP(
                    tensor=cs_d_ap.tensor, offset=cs_d_ap.offset,
                    ap=[[0, DH]] + list(cs_d_ap.ap[1:])))
                nc.vector.tensor_mul(out=dst, in0=otus, in1=recs)

        def outproj_rs(s):
            sbB = pools["sbB"]
            # partial out-projection for this slice's 4 row blocks
            for it4 in range(4):
                it = s * 4 + it4
                op_sb = sbB.tile([128, D], FP, tag="op", name="op_sb")
                for nh in range(2):
                    op_ps = psP.tile([128, 512], FP, tag="ps", name="op_ps")
                    for kt in range(2):
                        nc.tensor.matmul(op_ps, OT[kt][:, it * 128:(it + 1) * 128],
                                         wo_sb[:, kt, nh * 512:(nh + 1) * 512],
                                         start=(kt == 0), stop=(kt == 1))
                    nc.vector.tensor_copy(out=op_sb[:, nh * 512:(nh + 1) * 512],
                                          in_=op_ps)
                nc.sync.dma_start(
                    out=oph[s][:].rearrange("(t p) d -> t p d", p=128)[it4],
                    in_=op_sb)
            nc.gpsimd.collective_compute(
                "ReduceScatter", ALU.add,
                replica_groups=[[0, 1, 2, 3], [4, 5, 6, 7]],
                ins=[oph[s].opt()], outs=[rsh[s].opt()])

        def ln_tail(s):
            # residual + LayerNorm on this core's 128-row chunk of slice s.
            # Ops are spread over gpsimd / vector / scalar so no single queue
            # serializes the tail; stats come from the fused accumulators.
            sbB = pools["sbB"]
            xr = sbB.tile([128, D], FP, tag=f"xr{s}", name=f"xr{s}")
            rd = sbB.tile([128, D], FP, tag=f"rd{s}", name=f"rd{s}")
            rs_sb = sbB.tile([128, D], FP, tag=f"rs{s}", name=f"rs{s}")
            nc.sync.dma_start(out=rd, in_=res_tiles[s])
            nc.sync.dma_start(out=rs_sb, in_=rsh[s][:])
            nc.gpsimd.tensor_add(out=xr, in0=rs_sb, in1=rd)
            # mean/var via bn_stats; rstd = sqrt(1/(var+eps)) so the scalar
            # engine only ever holds the Exp and Sqrt tables (no thrash)
            stats = sbB.tile([128, 2, 6], FP, tag=f"st{s}", name=f"st{s}")
            mv = sbB.tile([128, 4], FP, tag=f"mv{s}", name=f"mv{s}")
            nc.vector.bn_stats(out=stats[:, 0, :], in_=xr[:, 0:512])
            nc.vector.bn_stats(out=stats[:, 1, :], in_=xr[:, 512:1024])
            nc.vector.bn_aggr(out=mv[:, 0:2], in_=stats)
            nc.vector.tensor_scalar(out=mv[:, 1:2], in0=mv[:, 1:2],
                                    scalar1=LN_EPS, scalar2=None, op0=ALU.add)
            nc.vector.reciprocal(out=mv[:, 2:3], in_=mv[:, 1:2])
            nc.scalar.sqrt(out=mv[:, 3:4], in_=mv[:, 2:3])
            nc.vector.tensor_scalar(out=xr, in0=xr,
                                    scalar1=mv[:, 0:1], scalar2=mv[:, 3:4],
                                    op0=ALU.subtract, op1=ALU.mult)
            nc.gpsimd.tensor_mul(out=xr, in0=xr, in1=g_sb)
            nc.vector.tensor_add(out=xr, in0=xr, in1=b_sb)
            nc.sync.dma_start(out=out_tiles[s], in_=xr)

        for s in range(NS):
            attention(s, 0)
            if s == 0:
                proj_qkp(1)  # overlaps first attention slice on other engines
                # x/pos/weight staging no longer needed; free its SBUF before
                # opening the out-proj/LN pool
                ph12_ctx.close()
                pools["sbB"] = ctx.enter_context(tc.tile_pool(name="sbB", bufs=1))
            else:
                outproj_rs(s - 1)  # issued mid-slice: PE never starves on it
            attention(s, 1)
        outproj_rs(NS - 1)
        for s in range(NS):
            ln_tail(s)

    nc.compile()
    return nc


_NC = None
_last_in_maps = None


def kernel(**inputs) -> np.ndarray:
    global _NC, _last_in_maps
    if _NC is None:
        _NC = build()
    nc = _NC

    q_s = np.asarray(inputs["q_s"], np.float32)
    pos = np.asarray(inputs["pos_emb"], np.float32)
    Wq = np.asarray(inputs["Wq"], np.float32)
    Wk = np.asarray(inputs["Wk"], np.float32)
    Wv = np.asarray(inputs["Wv"], np.float32)
    Wo = np.asarray(inputs["Wo"], np.float32)
    bo = np.asarray(inputs["bo"], np.float32)
    ln_g = np.asarray(inputs["ln_g"], np.float32)
    ln_b = np.asarray(inputs["ln_b"], np.float32)

    in_maps = []
    for c in range(NCORES):
        b, g = divmod(c, GRP)
        cs = slice(g * C, (g + 1) * C)
        resid = np.concatenate(
            [q_s[b][512 * s + 128 * g: 512 * s + 128 * (g + 1)] for s in range(NS)],
            axis=0) + bo[None, :]
        bf = ml_dtypes.bfloat16
        in_maps.append({
            "xT": np.ascontiguousarray(q_s[b].T.astype(bf)),
            "posT": np.ascontiguousarray(pos[b][:, cs].T),
            "wq": np.ascontiguousarray(Wq[:, cs].astype(bf)),
            "wk": np.ascontiguousarray(Wk[:, cs].astype(bf)),
            "wv": np.ascontiguousarray(Wv[:, cs].astype(bf)),
            "wo": np.ascontiguousarray(Wo[cs, :].astype(bf)),
            "resid": np.ascontiguousarray(resid),
            "ln_g": ln_g,
            "ln_b": ln_b,
        })

    _last_in_maps = in_maps
    res = run_bass_kernel_spmd(nc, in_maps, list(range(NCORES)))
    out = np.empty((B, N, D), np.float32)
    for c in range(NCORES):
        b, g = divmod(c, GRP)
        o = res.results[c]["out"]
        for s in range(NS):
            out[b, 512 * s + 128 * g: 512 * s + 128 * (g + 1), :] = \
                o[128 * s:128 * (s + 1)]
    return out


# revision 15
# speedup vs baseline: 1.0433x; 1.0433x over previous
"""BoT multi-head attention block (QKV proj + content/position attention +
out-proj + residual + LayerNorm) on 8 Trainium2 NeuronCores.

Sharding: tensor-parallel over heads (4 heads/core) x batch (2 batches, 4
cores each).  Each core computes q/k/v projections for its 256 feature
columns, full attention for its 4 heads, and a partial out-projection;
partials are summed with per-slice ReduceScatters over each 4-core batch
group (overlapped with attention of later slices), after which each core
applies residual + LayerNorm to its 4x128 rows.

Layout trick: attention logits are computed TRANSPOSED (j on partitions, i
free) so the softmax numerator matmul (P^T moving, V stationary) needs no
transpose of the probability matrix; an extra all-ones column in the
stationary V supplies the softmax denominator for free.  Host passes x and
pos pre-transposed.

Pipeline: residual+LayerNorm runs in a tail phase (never blocking the
in-order Vector queue mid-attention); out-proj for slice s is issued
between the two head-pair attention passes of slice s+1 so the PE queue
always has independent work while ReduceScatter s is in flight; a tiny
warm-up collective at t=0 absorbs the CC barrier + link warm-up.
"""

import contextlib
import os
import sys

os.environ.setdefault("MYCRO_LOCAL_CACHE", "1")
for _p in ("/opt/trn_rl_repo",):
    if os.path.isdir(_p) and _p not in sys.path:
        sys.path.append(_p)

import ml_dtypes
import numpy as np

import concourse.bass as bass
from concourse import bacc
import concourse.mybir as mybir
import concourse.tile as tile
from concourse.bass_utils import run_bass_kernel_spmd

FP = mybir.dt.float32
BF = mybir.dt.bfloat16
AF = mybir.ActivationFunctionType
ALU = mybir.AluOpType

B, N, D, H = 2, 2048, 1024, 16
NCORES = 8
GRP = 4                # cores per batch group
HPC = H // GRP         # heads per core = 4
C = D // GRP           # feature cols per core = 256
R = N // GRP           # output rows per core = 512
DH = D // H            # head dim = 64
SCALE = DH ** -0.5
LN_EPS = 1e-5

NT = N // 128          # 16 row tiles
KD = D // 128          # 8 contraction tiles over D
NS = N // 512          # 4 i-slices

ATT_DT = BF            # dtype of attention matmul operands
PROJ_DT = BF           # dtype of projection inputs (xT, wq/wk/wv)


def build():
    nc = bacc.Bacc("TRN2", target_bir_lowering=False, num_devices=NCORES)

    xT_t = nc.dram_tensor("xT", [D, N], PROJ_DT, kind="ExternalInput")
    posT_t = nc.dram_tensor("posT", [C, N], FP, kind="ExternalInput")
    wq_t = nc.dram_tensor("wq", [D, C], PROJ_DT, kind="ExternalInput")
    wk_t = nc.dram_tensor("wk", [D, C], PROJ_DT, kind="ExternalInput")
    wv_t = nc.dram_tensor("wv", [D, C], PROJ_DT, kind="ExternalInput")
    wo_t = nc.dram_tensor("wo", [C, D], BF, kind="ExternalInput")
    res_t = nc.dram_tensor("resid", [R, D], FP, kind="ExternalInput")
    g_t = nc.dram_tensor("ln_g", [D], FP, kind="ExternalInput")
    bt_t = nc.dram_tensor("ln_b", [D], FP, kind="ExternalInput")
    out_t = nc.dram_tensor("out", [R, D], FP, kind="ExternalOutput")

    res_tiles = res_t.ap().rearrange("(t p) d -> t p d", p=128)
    out_tiles = out_t.ap().rearrange("(t p) d -> t p d", p=128)

    def bcast_ap(ap, parts):
        return bass.AP(tensor=ap.tensor, offset=ap.offset,
                       ap=[[0, parts]] + list(ap.ap))

    with tile.TileContext(nc) as tc, contextlib.ExitStack() as ctx:
        persist = ctx.enter_context(tc.tile_pool(name="persist", bufs=1))
        attnp = ctx.enter_context(tc.tile_pool(name="attnp", bufs=1))
        psP = ctx.enter_context(tc.tile_pool(name="psP", bufs=1, space="PSUM"))
        psO = ctx.enter_context(tc.tile_pool(name="psO", bufs=3, space="PSUM"))
        psC = ctx.enter_context(tc.tile_pool(name="psC", bufs=2, space="PSUM"))
        dram = ctx.enter_context(tc.tile_pool(name="dram", bufs=1, space="DRAM"))

        ones64 = persist.tile([1, DH], FP, tag="ones64")
        nc.vector.memset(ones64, 1.0)
        onescol = persist.tile([128, 1], FP, tag="onescol")
        nc.vector.memset(onescol, 1.0)

        sbA = ctx.enter_context(tc.tile_pool(name="sbA", bufs=3))

        # ---------------- phase 1-2: load (pre-transposed on host), project
        ph12_ctx = contextlib.ExitStack()
        p12 = ph12_ctx.enter_context(tc.tile_pool(name="ph12", bufs=1))

        wq_sb = p12.tile([128, KD, C], PROJ_DT, tag="wq")
        wk_sb = p12.tile([128, KD, C], PROJ_DT, tag="wk")
        wv_sb = p12.tile([128, KD, C], PROJ_DT, tag="wv")
        xT_sb = p12.tile([128, KD, N], PROJ_DT, tag="xT")
        xT_src = xT_t.ap().rearrange("(k p) n -> p k n", p=128)
        wq_src = wq_t.ap().rearrange("(k p) c -> p k c", p=128)
        # interleave per-k chunks so the first projection matmul can start
        # as soon as wq[0]/xT[0] land instead of after the full staging DMA
        for k in range(KD):
            nc.sync.dma_start(out=wq_sb[:, k, :], in_=wq_src[:, k, :])
            nc.sync.dma_start(out=xT_sb[:, k, :], in_=xT_src[:, k, :])
        nc.sync.dma_start(out=wk_sb, in_=wk_t.ap().rearrange("(k p) c -> p k c", p=128))
        posT_sb = p12.tile([128, 2, N], FP, tag="posT")
        nc.sync.dma_start(out=posT_sb,
                          in_=posT_t.ap().rearrange("(m p) n -> p m n", p=128))
        nc.sync.dma_start(out=wv_sb, in_=wv_t.ap().rearrange("(k p) c -> p k c", p=128))
        xT = [xT_sb[:, k, :] for k in range(KD)]
        posT = [posT_sb[:, m, :] for m in range(2)]

        wo_sb = persist.tile([128, 2, D], BF, tag="wo")
        nc.sync.dma_start(out=wo_sb, in_=wo_t.ap().rearrange("(k p) d -> p k d", p=128))
        g_sb = persist.tile([128, D], FP, tag="g")
        b_sb = persist.tile([128, D], FP, tag="b")
        nc.gpsimd.dma_start(out=g_sb, in_=bcast_ap(g_t.ap(), 128))
        nc.gpsimd.dma_start(out=b_sb, in_=bcast_ap(bt_t.ap(), 128))

        # projections: qT/kpT [128 c, N] (head pair hp at rows 64*(h%2))
        qT = [attnp.tile([128, N], ATT_DT, name=f"qT{m}", tag=f"qT{m}") for m in range(2)]
        kpT = [attnp.tile([128, N], ATT_DT, name=f"kpT{m}", tag=f"kpT{m}") for m in range(2)]
        V = [attnp.tile([128, HPC, DH + 1], ATT_DT, name=f"V{t}", tag=f"V{t}")
             for t in range(NT)]

        def proj_qkp(m):
            for s in range(NS):
                q_ps = psP.tile([128, 512], FP, tag="ps", name="q_ps")
                for k in range(KD):
                    nc.tensor.matmul(q_ps, wq_sb[:, k, m * 128:(m + 1) * 128],
                                     xT[k][:, s * 512:(s + 1) * 512],
                                     start=(k == 0), stop=(k == KD - 1))
                nc.vector.tensor_copy(out=qT[m][:, s * 512:(s + 1) * 512], in_=q_ps)
            for s in range(NS):
                kp_ps = psP.tile([128, 512], FP, tag="ps", name="kp_ps")
                for k in range(KD):
                    nc.tensor.matmul(kp_ps, wk_sb[:, k, m * 128:(m + 1) * 128],
                                     xT[k][:, s * 512:(s + 1) * 512],
                                     start=(k == 0), stop=(k == KD - 1))
                nc.vector.tensor_add(out=kpT[m][:, s * 512:(s + 1) * 512],
                                     in0=kp_ps, in1=posT[m][:, s * 512:(s + 1) * 512])

        proj_qkp(0)
        for t in range(NT):
            v_ps = psP.tile([128, C], FP, tag="ps", name="v_ps")
            for k in range(KD):
                nc.tensor.matmul(v_ps, xT[k][:, t * 128:(t + 1) * 128], wv_sb[:, k, :],
                                 start=(k == 0), stop=(k == KD - 1))
            nc.vector.tensor_copy(out=V[t][:, :, 0:DH],
                                  in_=v_ps.rearrange("p (h d) -> p h d", h=HPC))
            nc.vector.tensor_copy(out=V[t][:, :, DH:DH + 1],
                                  in_=onescol.broadcast_to([128, HPC, 1]))

        # ---------------- phases 3-5: attention / out-proj+RS / tail LN ----
        pools = {}

        # unnormalized attention output, bf16, normalized in place on gpsimd
        OT = [attnp.tile([128, N], BF, name=f"OT{m}", tag=f"OT{m}") for m in range(2)]
        oph = [dram.tile([R, D], FP, name=f"oph{s}", tag=f"oph{s}") for s in range(NS)]
        rsh = [dram.tile([128, D], FP, name=f"rsh{s}", tag=f"rsh{s}") for s in range(NS)]

        def attention(s, hp):
            ot_e = psO.tile([128, 512], FP, tag="ot", name="ot_e")
            ot_o = psO.tile([128, 512], FP, tag="ot", name="ot_o")
            for jt in range(NT):
                st = psC.tile([128, 1024], FP, tag="st", name="st")
                nc.tensor.matmul(st[:, 0:512],
                                 kpT[hp][0:64, jt * 128:(jt + 1) * 128],
                                 qT[hp][0:64, s * 512:(s + 1) * 512],
                                 start=True, stop=True)
                nc.tensor.matmul(st[:, 512:1024],
                                 kpT[hp][64:128, jt * 128:(jt + 1) * 128],
                                 qT[hp][64:128, s * 512:(s + 1) * 512],
                                 start=True, stop=True)
                ste = sbA.tile([128, 1024], ATT_DT, tag="ste", name="ste")
                nc.scalar.activation(out=ste, in_=st, func=AF.Exp, scale=SCALE)
                nc.tensor.matmul(ot_e[0:DH + 1, :], V[jt][:, 2 * hp, :],
                                 ste[:, 0:512],
                                 start=(jt == 0), stop=(jt == NT - 1))
                nc.tensor.matmul(ot_o[0:DH + 1, :], V[jt][:, 2 * hp + 1, :],
                                 ste[:, 512:1024],
                                 start=(jt == 0), stop=(jt == NT - 1))
            # evacuate PSUM: reciprocal of the colsum row straight from PSUM,
            # unnormalized rows to fp32 staging; softmax division writes the
            # bf16 OT used as the out-proj stationary
            for par, ot in ((0, ot_e), (1, ot_o)):
                csrow = sbA.tile([1, 512], FP, tag="csrow", name="csrow", bufs=8)
                nc.vector.tensor_copy(out=csrow, in_=ot[DH:DH + 1, :])
                csr = sbA.tile([1, 512], FP, tag="csr", name="csr", bufs=4)
                nc.vector.reciprocal_approx_fast(out=csr, in_=csrow)
                otu = sbA.tile([128, 512], FP, tag="otu", name="otu", bufs=4)
                otus = otu[par * 64:par * 64 + DH, :]
                nc.vector.tensor_copy(out=otus, in_=ot[0:DH, :])
                dst = OT[hp][par * 64:par * 64 + DH, s * 512:(s + 1) * 512]
                cs_d = dram.tile([1, 512], FP, tag="cs_d", name="cs_d", bufs=4)
                nc.sync.dma_start(out=cs_d[:], in_=csr)
                # rec must share its base partition with dst (DVE 2-SBUF rule)
                rec = sbA.tile([128, 512], FP, tag="rec", name="rec", bufs=4)
                recs = rec[par * 64:par * 64 + DH, :]
                cs_d_ap = cs_d.opt()
                nc.gpsimd.dma_start(out=recs, in_=bass.AP(
                    tensor=cs_d_ap.tensor, offset=cs_d_ap.offset,
                    ap=[[0, DH]] + list(cs_d_ap.ap[1:])))
                nc.vector.tensor_mul(out=dst, in0=otus, in1=recs)

        def outproj_rs(s):
            sbB = pools["sbB"]
            # partial out-projection for this slice's 4 row blocks
            for it4 in range(4):
                it = s * 4 + it4
                op_sb = sbB.tile([128, D], FP, tag="op", name="op_sb")
                for nh in range(2):
                    op_ps = psP.tile([128, 512], FP, tag="ps", name="op_ps")
                    for kt in range(2):
                        nc.tensor.matmul(op_ps, OT[kt][:, it * 128:(it + 1) * 128],
                                         wo_sb[:, kt, nh * 512:(nh + 1) * 512],
                                         start=(kt == 0), stop=(kt == 1))
                    nc.vector.tensor_copy(out=op_sb[:, nh * 512:(nh + 1) * 512],
                                          in_=op_ps)
                nc.sync.dma_start(
                    out=oph[s][:].rearrange("(t p) d -> t p d", p=128)[it4],
                    in_=op_sb)
            nc.gpsimd.collective_compute(
                "ReduceScatter", ALU.add,
                replica_groups=[[0, 1, 2, 3], [4, 5, 6, 7]],
                ins=[oph[s].opt()], outs=[rsh[s].opt()])

        rd_tiles = {}

        def ln_prefetch(s):
            sbB = pools["sbB"]
            rd = sbB.tile([128, D], FP, tag=f"rd{s}", name=f"rd{s}")
            nc.sync.dma_start(out=rd, in_=res_tiles[s])
            rd_tiles[s] = rd

        def ln_tail(s):
            # residual + LayerNorm on this core's 128-row chunk of slice s.
            # Ops are spread over gpsimd / vector / scalar so no single queue
            # serializes the tail; stats come from the fused accumulators.
            sbB = pools["sbB"]
            xr = sbB.tile([128, D], FP, tag=f"xr{s}", name=f"xr{s}")
            rd = rd_tiles[s]
            rs_sb = sbB.tile([128, D], FP, tag=f"rs{s}", name=f"rs{s}")
            nc.sync.dma_start(out=rs_sb, in_=rsh[s][:])
            nc.gpsimd.tensor_add(out=xr, in0=rs_sb, in1=rd)
            # mean/var via bn_stats; rstd = sqrt(1/(var+eps)) so the scalar
            # engine only ever holds the Exp and Sqrt tables (no thrash)
            stats = sbB.tile([128, 2, 6], FP, tag=f"st{s}", name=f"st{s}")
            mv = sbB.tile([128, 4], FP, tag=f"mv{s}", name=f"mv{s}")
            nc.vector.bn_stats(out=stats[:, 0, :], in_=xr[:, 0:512])
            nc.vector.bn_stats(out=stats[:, 1, :], in_=xr[:, 512:1024])
            nc.vector.bn_aggr(out=mv[:, 0:2], in_=stats)
            nc.vector.tensor_scalar(out=mv[:, 1:2], in0=mv[:, 1:2],
                                    scalar1=LN_EPS, scalar2=None, op0=ALU.add)
            nc.vector.reciprocal(out=mv[:, 2:3], in_=mv[:, 1:2])
            nc.scalar.sqrt(out=mv[:, 3:4], in_=mv[:, 2:3])
            nc.vector.tensor_scalar(out=xr, in0=xr,
                                    scalar1=mv[:, 0:1], scalar2=mv[:, 3:4],
                                    op0=ALU.subtract, op1=ALU.mult)
            nc.gpsimd.tensor_mul(out=xr, in0=xr, in1=g_sb)
            nc.vector.tensor_add(out=xr, in0=xr, in1=b_sb)
            nc.sync.dma_start(out=out_tiles[s], in_=xr)

        for s in range(NS):
            attention(s, 0)
            if s == 0:
                proj_qkp(1)  # overlaps first attention slice on other engines
                # x/pos/weight staging no longer needed; free its SBUF before
                # opening the out-proj/LN pool
                ph12_ctx.close()
                pools["sbB"] = ctx.enter_context(tc.tile_pool(name="sbB", bufs=1))
            else:
                outproj_rs(s - 1)  # issued mid-slice: PE never starves on it
                ln_prefetch(s - 1)
            attention(s, 1)
        outproj_rs(NS - 1)
        ln_prefetch(NS - 1)
        # scheduler-only fence: LN work may never be reordered ahead of the
        # attention/out-proj stream (a hoisted RS-dependent op at the head
        # of an in-order queue stalls everything behind it)
        tc.no_sync_barrier()
        for s in range(NS):
            ln_tail(s)

    nc.compile()
    return nc


_NC = None
_last_in_maps = None


def kernel(**inputs) -> np.ndarray:
    global _NC, _last_in_maps
    if _NC is None:
        _NC = build()
    nc = _NC

    q_s = np.asarray(inputs["q_s"], np.float32)
    pos = np.asarray(inputs["pos_emb"], np.float32)
    Wq = np.asarray(inputs["Wq"], np.float32)
    Wk = np.asarray(inputs["Wk"], np.float32)
    Wv = np.asarray(inputs["Wv"], np.float32)
    Wo = np.asarray(inputs["Wo"], np.float32)
    bo = np.asarray(inputs["bo"], np.float32)
    ln_g = np.asarray(inputs["ln_g"], np.float32)
    ln_b = np.asarray(inputs["ln_b"], np.float32)

    in_maps = []
    for c in range(NCORES):
        b, g = divmod(c, GRP)
        cs = slice(g * C, (g + 1) * C)
        resid = np.concatenate(
            [q_s[b][512 * s + 128 * g: 512 * s + 128 * (g + 1)] for s in range(NS)],
            axis=0) + bo[None, :]
        bf = ml_dtypes.bfloat16
        in_maps.append({
            "xT": np.ascontiguousarray(q_s[b].T.astype(bf)),
            "posT": np.ascontiguousarray(pos[b][:, cs].T),
            "wq": np.ascontiguousarray(Wq[:, cs].astype(bf)),
            "wk": np.ascontiguousarray(Wk[:, cs].astype(bf)),
            "wv": np.ascontiguousarray(Wv[:, cs].astype(bf)),
            "wo": np.ascontiguousarray(Wo[cs, :].astype(bf)),
            "resid": np.ascontiguousarray(resid),
            "ln_g": ln_g,
            "ln_b": ln_b,
        })

    _last_in_maps = in_maps
    res = run_bass_kernel_spmd(nc, in_maps, list(range(NCORES)))
    out = np.empty((B, N, D), np.float32)
    for c in range(NCORES):
        b, g = divmod(c, GRP)
        o = res.results[c]["out"]
        for s in range(NS):
            out[b, 512 * s + 128 * g: 512 * s + 128 * (g + 1), :] = \
                o[128 * s:128 * (s + 1)]
    return out


# revision 19
# speedup vs baseline: 1.2355x; 1.1842x over previous
"""BoT multi-head attention block (QKV proj + content/position attention +
out-proj + residual + LayerNorm) on 8 Trainium2 NeuronCores.

Sharding: tensor-parallel over heads (4 heads/core) x batch (2 batches, 4
cores each).  Each core computes q/k/v projections for its 256 feature
columns, full attention for its 4 heads, and a partial out-projection;
partials are summed with per-slice ReduceScatters over each 4-core batch
group (overlapped with attention of later slices), after which each core
applies residual + LayerNorm to its 4x128 rows.

Layout trick: attention logits are computed TRANSPOSED (j on partitions, i
free) so the softmax numerator matmul (P^T moving, V stationary) needs no
transpose of the probability matrix; an extra all-ones column in the
stationary V supplies the softmax denominator for free.  Host passes x and
pos pre-transposed.

Pipeline: residual+LayerNorm runs in a tail phase (never blocking the
in-order Vector queue mid-attention); out-proj for slice s is issued
between the two head-pair attention passes of slice s+1 so the PE queue
always has independent work while ReduceScatter s is in flight; a tiny
warm-up collective at t=0 absorbs the CC barrier + link warm-up.
"""

import contextlib
import os
import sys

os.environ.setdefault("MYCRO_LOCAL_CACHE", "1")
for _p in ("/opt/trn_rl_repo",):
    if os.path.isdir(_p) and _p not in sys.path:
        sys.path.append(_p)

import ml_dtypes
import numpy as np

import concourse.bass as bass
from concourse import bacc
import concourse.mybir as mybir
import concourse.tile as tile
from concourse.bass_utils import run_bass_kernel_spmd

FP = mybir.dt.float32
BF = mybir.dt.bfloat16
AF = mybir.ActivationFunctionType
ALU = mybir.AluOpType

B, N, D, H = 2, 2048, 1024, 16
NCORES = 8
GRP = 4                # cores per batch group
HPC = H // GRP         # heads per core = 4
C = D // GRP           # feature cols per core = 256
R = N // GRP           # output rows per core = 512
DH = D // H            # head dim = 64
SCALE = DH ** -0.5
LN_EPS = 1e-5

NT = N // 128          # 16 row tiles
KD = D // 128          # 8 contraction tiles over D
NS = N // 512          # 4 i-slices

ATT_DT = BF            # dtype of attention matmul operands
PROJ_DT = BF           # dtype of projection inputs (xT, wq/wk/wv)


def build():
    nc = bacc.Bacc("TRN2", target_bir_lowering=False, num_devices=NCORES)

    xT_t = nc.dram_tensor("xT", [D, N], PROJ_DT, kind="ExternalInput")
    posT_t = nc.dram_tensor("posT", [C, N], FP, kind="ExternalInput")
    wq_t = nc.dram_tensor("wq", [D, C], PROJ_DT, kind="ExternalInput")
    wk_t = nc.dram_tensor("wk", [D, C], PROJ_DT, kind="ExternalInput")
    wv_t = nc.dram_tensor("wv", [D, C], PROJ_DT, kind="ExternalInput")
    wo_t = nc.dram_tensor("wo", [C, D], BF, kind="ExternalInput")
    res_t = nc.dram_tensor("resid", [R, D], FP, kind="ExternalInput")
    g_t = nc.dram_tensor("ln_g", [D], FP, kind="ExternalInput")
    bt_t = nc.dram_tensor("ln_b", [D], FP, kind="ExternalInput")
    out_t = nc.dram_tensor("out", [R, D], FP, kind="ExternalOutput")

    res_tiles = res_t.ap().rearrange("(t p) d -> t p d", p=128)
    out_tiles = out_t.ap().rearrange("(t p) d -> t p d", p=128)

    def bcast_ap(ap, parts):
        return bass.AP(tensor=ap.tensor, offset=ap.offset,
                       ap=[[0, parts]] + list(ap.ap))

    with tile.TileContext(nc) as tc, contextlib.ExitStack() as ctx:
        persist = ctx.enter_context(tc.tile_pool(name="persist", bufs=1))
        attnp = ctx.enter_context(tc.tile_pool(name="attnp", bufs=1))
        psP = ctx.enter_context(tc.tile_pool(name="psP", bufs=1, space="PSUM"))
        psO = ctx.enter_context(tc.tile_pool(name="psO", bufs=3, space="PSUM"))
        psC = ctx.enter_context(tc.tile_pool(name="psC", bufs=2, space="PSUM"))
        dram = ctx.enter_context(tc.tile_pool(name="dram", bufs=1, space="DRAM"))

        ones64 = persist.tile([1, DH], FP, tag="ones64")
        nc.vector.memset(ones64, 1.0)
        onescol = persist.tile([128, 1], FP, tag="onescol")
        nc.vector.memset(onescol, 1.0)

        sbA = ctx.enter_context(tc.tile_pool(name="sbA", bufs=3))

        # ---------------- phase 1-2: load (pre-transposed on host), project
        ph12_ctx = contextlib.ExitStack()
        p12 = ph12_ctx.enter_context(tc.tile_pool(name="ph12", bufs=1))

        wq_sb = p12.tile([128, KD, C], PROJ_DT, tag="wq")
        wk_sb = p12.tile([128, KD, C], PROJ_DT, tag="wk")
        wv_sb = p12.tile([128, KD, C], PROJ_DT, tag="wv")
        xT_sb = p12.tile([128, KD, N], PROJ_DT, tag="xT")
        xT_src = xT_t.ap().rearrange("(k p) n -> p k n", p=128)
        wq_src = wq_t.ap().rearrange("(k p) c -> p k c", p=128)
        # interleave per-k chunks so the first projection matmul can start
        # as soon as wq[0]/xT[0] land instead of after the full staging DMA
        for k in range(KD):
            nc.sync.dma_start(out=wq_sb[:, k, :], in_=wq_src[:, k, :])
            nc.sync.dma_start(out=xT_sb[:, k, :], in_=xT_src[:, k, :])
        nc.sync.dma_start(out=wk_sb, in_=wk_t.ap().rearrange("(k p) c -> p k c", p=128))
        posT_sb = p12.tile([128, 2, N], FP, tag="posT")
        nc.sync.dma_start(out=posT_sb,
                          in_=posT_t.ap().rearrange("(m p) n -> p m n", p=128))
        nc.sync.dma_start(out=wv_sb, in_=wv_t.ap().rearrange("(k p) c -> p k c", p=128))
        xT = [xT_sb[:, k, :] for k in range(KD)]
        posT = [posT_sb[:, m, :] for m in range(2)]

        wo_sb = persist.tile([128, 2, D], BF, tag="wo")
        nc.sync.dma_start(out=wo_sb, in_=wo_t.ap().rearrange("(k p) d -> p k d", p=128))
        g_sb = persist.tile([128, D], FP, tag="g")
        b_sb = persist.tile([128, D], FP, tag="b")
        nc.gpsimd.dma_start(out=g_sb, in_=bcast_ap(g_t.ap(), 128))
        nc.gpsimd.dma_start(out=b_sb, in_=bcast_ap(bt_t.ap(), 128))

        # projections: qT/kpT [128 c, N] (head pair hp at rows 64*(h%2))
        qT = [attnp.tile([128, N], ATT_DT, name=f"qT{m}", tag=f"qT{m}") for m in range(2)]
        kpT = [attnp.tile([128, N], ATT_DT, name=f"kpT{m}", tag=f"kpT{m}") for m in range(2)]
        V = [attnp.tile([128, HPC, DH + 1], ATT_DT, name=f"V{t}", tag=f"V{t}")
             for t in range(NT)]

        def proj_qkp(m):
            for s in range(NS):
                q_ps = psP.tile([128, 512], FP, tag="ps", name="q_ps")
                for k in range(KD):
                    nc.tensor.matmul(q_ps, wq_sb[:, k, m * 128:(m + 1) * 128],
                                     xT[k][:, s * 512:(s + 1) * 512],
                                     start=(k == 0), stop=(k == KD - 1))
                nc.vector.tensor_copy(out=qT[m][:, s * 512:(s + 1) * 512], in_=q_ps)
            for s in range(NS):
                kp_ps = psP.tile([128, 512], FP, tag="ps", name="kp_ps")
                for k in range(KD):
                    nc.tensor.matmul(kp_ps, wk_sb[:, k, m * 128:(m + 1) * 128],
                                     xT[k][:, s * 512:(s + 1) * 512],
                                     start=(k == 0), stop=(k == KD - 1))
                nc.vector.tensor_add(out=kpT[m][:, s * 512:(s + 1) * 512],
                                     in0=kp_ps, in1=posT[m][:, s * 512:(s + 1) * 512])

        proj_qkp(0)
        for t in range(NT):
            v_ps = psP.tile([128, C], FP, tag="ps", name="v_ps")
            for k in range(KD):
                nc.tensor.matmul(v_ps, xT[k][:, t * 128:(t + 1) * 128], wv_sb[:, k, :],
                                 start=(k == 0), stop=(k == KD - 1))
            nc.vector.tensor_copy(out=V[t][:, :, 0:DH],
                                  in_=v_ps.rearrange("p (h d) -> p h d", h=HPC))
            nc.vector.tensor_copy(out=V[t][:, :, DH:DH + 1],
                                  in_=onescol.broadcast_to([128, HPC, 1]))

        # ---------------- phases 3-5: attention / out-proj+RS / tail LN ----
        pools = {}

        # unnormalized attention output, bf16, normalized in place on gpsimd
        OT = [attnp.tile([128, N], BF, name=f"OT{m}", tag=f"OT{m}") for m in range(2)]
        # out-proj partials travel over the 4-core ReduceScatter in bf16:
        # halves both the collective time and the staging DMA volume
        oph = [dram.tile([R, D], BF, name=f"oph{s}", tag=f"oph{s}") for s in range(NS)]
        rsh = [dram.tile([128, D], BF, name=f"rsh{s}", tag=f"rsh{s}") for s in range(NS)]

        def attention(s, hp):
            ot_e = psO.tile([128, 512], FP, tag="ot", name="ot_e")
            ot_o = psO.tile([128, 512], FP, tag="ot", name="ot_o")
            for jt in range(NT):
                st = psC.tile([128, 1024], FP, tag="st", name="st")
                nc.tensor.matmul(st[:, 0:512],
                                 kpT[hp][0:64, jt * 128:(jt + 1) * 128],
                                 qT[hp][0:64, s * 512:(s + 1) * 512],
                                 start=True, stop=True)
                nc.tensor.matmul(st[:, 512:1024],
                                 kpT[hp][64:128, jt * 128:(jt + 1) * 128],
                                 qT[hp][64:128, s * 512:(s + 1) * 512],
                                 start=True, stop=True)
                ste = sbA.tile([128, 1024], ATT_DT, tag="ste", name="ste")
                nc.scalar.activation(out=ste, in_=st, func=AF.Exp, scale=SCALE)
                nc.tensor.matmul(ot_e[0:DH + 1, :], V[jt][:, 2 * hp, :],
                                 ste[:, 0:512],
                                 start=(jt == 0), stop=(jt == NT - 1))
                nc.tensor.matmul(ot_o[0:DH + 1, :], V[jt][:, 2 * hp + 1, :],
                                 ste[:, 512:1024],
                                 start=(jt == 0), stop=(jt == NT - 1))
            # evacuate PSUM: reciprocal of the colsum row straight from PSUM,
            # unnormalized rows to fp32 staging; softmax division writes the
            # bf16 OT used as the out-proj stationary
            for par, ot in ((0, ot_e), (1, ot_o)):
                csrow = sbA.tile([1, 512], FP, tag="csrow", name="csrow", bufs=8)
                nc.vector.tensor_copy(out=csrow, in_=ot[DH:DH + 1, :])
                csr = sbA.tile([1, 512], FP, tag="csr", name="csr", bufs=4)
                nc.vector.reciprocal_approx_fast(out=csr, in_=csrow)
                otu = sbA.tile([128, 512], FP, tag="otu", name="otu", bufs=4)
                otus = otu[par * 64:par * 64 + DH, :]
                nc.vector.tensor_copy(out=otus, in_=ot[0:DH, :])
                dst = OT[hp][par * 64:par * 64 + DH, s * 512:(s + 1) * 512]
                cs_d = dram.tile([1, 512], FP, tag="cs_d", name="cs_d", bufs=4)
                nc.sync.dma_start(out=cs_d[:], in_=csr)
                # rec must share its base partition with dst (DVE 2-SBUF rule)
                rec = sbA.tile([128, 512], FP, tag="rec", name="rec", bufs=4)
                recs = rec[par * 64:par * 64 + DH, :]
                cs_d_ap = cs_d.opt()
                nc.gpsimd.dma_start(out=recs, in_=bass.AP(
                    tensor=cs_d_ap.tensor, offset=cs_d_ap.offset,
                    ap=[[0, DH]] + list(cs_d_ap.ap[1:])))
                nc.vector.tensor_mul(out=dst, in0=otus, in1=recs)

        def outproj_rs(s):
            sbB = pools["sbB"]
            # partial out-projection for this slice's 4 row blocks
            for it4 in range(4):
                it = s * 4 + it4
                op_sb = sbB.tile([128, D], BF, tag="op", name="op_sb")
                for nh in range(2):
                    op_ps = psP.tile([128, 512], FP, tag="ps", name="op_ps")
                    for kt in range(2):
                        nc.tensor.matmul(op_ps, OT[kt][:, it * 128:(it + 1) * 128],
                                         wo_sb[:, kt, nh * 512:(nh + 1) * 512],
                                         start=(kt == 0), stop=(kt == 1))
                    nc.vector.tensor_copy(out=op_sb[:, nh * 512:(nh + 1) * 512],
                                          in_=op_ps)
                nc.sync.dma_start(
                    out=oph[s][:].rearrange("(t p) d -> t p d", p=128)[it4],
                    in_=op_sb)
            nc.gpsimd.collective_compute(
                "ReduceScatter", ALU.add,
                replica_groups=[[0, 1, 2, 3], [4, 5, 6, 7]],
                ins=[oph[s].opt()], outs=[rsh[s].opt()])

        rd_tiles = {}

        def ln_prefetch(s):
            sbB = pools["sbB"]
            rd = sbB.tile([128, D], FP, tag=f"rd{s}", name=f"rd{s}")
            nc.sync.dma_start(out=rd, in_=res_tiles[s])
            rd_tiles[s] = rd

        def ln_tail(s):
            # residual + LayerNorm on this core's 128-row chunk of slice s.
            # The [128,1024] elementwise steps are split into column halves
            # on vector/gpsimd so the serial tail chain is halved; rstd =
            # sqrt(1/(var+eps)) keeps the scalar engine on Exp+Sqrt tables.
            sbB = pools["sbB"]
            xr = sbB.tile([128, D], FP, tag=f"xr{s}", name=f"xr{s}")
            rd = rd_tiles[s]
            rs_sb = sbB.tile([128, D], BF, tag=f"rs{s}", name=f"rs{s}")
            rs32 = sbB.tile([128, D], FP, tag=f"r32{s}", name=f"r32{s}")
            nc.sync.dma_start(out=rs_sb, in_=rsh[s][:])
            h0, h1 = slice(0, 512), slice(512, 1024)
            nc.vector.tensor_copy(out=rs32[:, h0], in_=rs_sb[:, h0])
            nc.gpsimd.tensor_copy(out=rs32[:, h1], in_=rs_sb[:, h1])
            nc.vector.tensor_add(out=xr[:, h0], in0=rs32[:, h0], in1=rd[:, h0])
            nc.gpsimd.tensor_add(out=xr[:, h1], in0=rs32[:, h1], in1=rd[:, h1])
            stats = sbB.tile([128, 2, 6], FP, tag=f"st{s}", name=f"st{s}")
            mv = sbB.tile([128, 8], FP, tag=f"mv{s}", name=f"mv{s}")
            nc.vector.bn_stats(out=stats[:, 0, :], in_=xr[:, h0])
            nc.vector.bn_stats(out=stats[:, 1, :], in_=xr[:, h1])
            nc.vector.bn_aggr(out=mv[:, 0:2], in_=stats)
            nc.vector.tensor_scalar(out=mv[:, 1:2], in0=mv[:, 1:2],
                                    scalar1=LN_EPS, scalar2=None, op0=ALU.add)
            nc.vector.reciprocal(out=mv[:, 2:3], in_=mv[:, 1:2])
            nc.scalar.sqrt(out=mv[:, 3:4], in_=mv[:, 2:3])
            nc.vector.tensor_scalar(out=xr[:, h0], in0=xr[:, h0],
                                    scalar1=mv[:, 0:1], scalar2=mv[:, 3:4],
                                    op0=ALU.subtract, op1=ALU.mult)
            nc.vector.tensor_scalar(out=xr[:, h1], in0=xr[:, h1],
                                    scalar1=mv[:, 0:1], scalar2=mv[:, 3:4],
                                    op0=ALU.subtract, op1=ALU.mult)
            nc.vector.tensor_mul(out=xr[:, h0], in0=xr[:, h0], in1=g_sb[:, h0])
            nc.gpsimd.tensor_mul(out=xr[:, h1], in0=xr[:, h1], in1=g_sb[:, h1])
            nc.vector.tensor_add(out=xr[:, h0], in0=xr[:, h0], in1=b_sb[:, h0])
            nc.gpsimd.tensor_add(out=xr[:, h1], in0=xr[:, h1], in1=b_sb[:, h1])
            nc.sync.dma_start(out=out_tiles[s], in_=xr)

        for s in range(NS):
            attention(s, 0)
            if s == 0:
                proj_qkp(1)  # overlaps first attention slice on other engines
                # x/pos/weight staging no longer needed; free its SBUF before
                # opening the out-proj/LN pool
                ph12_ctx.close()
                pools["sbB"] = ctx.enter_context(tc.tile_pool(name="sbB", bufs=1))
            else:
                outproj_rs(s - 1)  # issued mid-slice: PE never starves on it
                ln_prefetch(s - 1)
            attention(s, 1)
        outproj_rs(NS - 1)
        ln_prefetch(NS - 1)
        # scheduler-only fence: LN work may never be reordered ahead of the
        # attention/out-proj stream (a hoisted RS-dependent op at the head
        # of an in-order queue stalls everything behind it)
        tc.no_sync_barrier()
        for s in range(NS):
            ln_tail(s)

    nc.compile()
    return nc


_NC = None
_last_in_maps = None


def kernel(**inputs) -> np.ndarray:
    global _NC, _last_in_maps
    if _NC is None:
        _NC = build()
    nc = _NC

    q_s = np.asarray(inputs["q_s"], np.float32)
    pos = np.asarray(inputs["pos_emb"], np.float32)
    Wq = np.asarray(inputs["Wq"], np.float32)
    Wk = np.asarray(inputs["Wk"], np.float32)
    Wv = np.asarray(inputs["Wv"], np.float32)
    Wo = np.asarray(inputs["Wo"], np.float32)
    bo = np.asarray(inputs["bo"], np.float32)
    ln_g = np.asarray(inputs["ln_g"], np.float32)
    ln_b = np.asarray(inputs["ln_b"], np.float32)

    in_maps = []
    for c in range(NCORES):
        b, g = divmod(c, GRP)
        cs = slice(g * C, (g + 1) * C)
        resid = np.concatenate(
            [q_s[b][512 * s + 128 * g: 512 * s + 128 * (g + 1)] for s in range(NS)],
            axis=0) + bo[None, :]
        bf = ml_dtypes.bfloat16
        in_maps.append({
            "xT": np.ascontiguousarray(q_s[b].T.astype(bf)),
            "posT": np.ascontiguousarray(pos[b][:, cs].T),
            "wq": np.ascontiguousarray(Wq[:, cs].astype(bf)),
            "wk": np.ascontiguousarray(Wk[:, cs].astype(bf)),
            "wv": np.ascontiguousarray(Wv[:, cs].astype(bf)),
            "wo": np.ascontiguousarray(Wo[cs, :].astype(bf)),
            "resid": np.ascontiguousarray(resid),
            "ln_g": ln_g,
            "ln_b": ln_b,
        })

    _last_in_maps = in_maps
    res = run_bass_kernel_spmd(nc, in_maps, list(range(NCORES)))
    out = np.empty((B, N, D), np.float32)
    for c in range(NCORES):
        b, g = divmod(c, GRP)
        o = res.results[c]["out"]
        for s in range(NS):
            out[b, 512 * s + 128 * g: 512 * s + 128 * (g + 1), :] = \
                o[128 * s:128 * (s + 1)]
    return out
